# revision 27
# baseline (speedup 1.0000x reference)
"""Bass/Trainium2 kernel for DynamicGraphConv (GNN message passing).

Computes, for a graph with N nodes / E edges:
    ns  = segment_sum(x[row], col, N)            # scatter-add of source features
    h   = concat([x, ns], -1) @ W + b
    out = LayerNorm(h) * gamma + beta

Distribution: nodes (and segment targets) are sharded across 8 NeuronCores;
edges are partitioned by destination-node shard so aggregation is local to
each core; the full x is replicated to every core's DRAM for the
source-feature gather (host-side "all-gather").

Per-core pipeline:
  - dma_gather streams x[row] for the core's edges into SBUF (messages);
    int16 indices are handled via 4 source-row buckets of 25k rows, with the
    core's edges sorted by (bucket, dest-window, src).
  - per 128-destination window, a one-hot matrix U[msg, slot] is built with
    one DVE is_equal op; PE accumulates nsT[64, 128] = sum over msg tiles of
    msgs^T @ U in PSUM (the scatter-add expressed as matmuls).
  - h = [x;1]^T @ [W1;b] + nsT^T @ W2 fused into one PSUM tile per window;
    LayerNorm is applied batched over groups of 8 windows.
"""

import sys

sys.path.insert(0, "/opt/trn_rl_repo")

import numpy as np

# ---- problem constants (hardcoded per contract) ----
N_NODES = 100000
N_EDGES = 1000000
D = 64
OUT = 64
EPS = 1e-5
N_CORES = 8

S = N_NODES // N_CORES          # nodes per core = 12500
WIN = 128                       # dest window size
NWIN = (S + WIN - 1) // WIN     # windows per core = 98
NBUCKET = 4
BR = (N_NODES + NBUCKET - 1) // NBUCKET   # source rows per bucket (int16-addressable)
CHUNK_TILES = 16                # msg tiles per gather call (2048 idx)
HBATCH = 8                      # windows per LayerNorm batch


# --------------------------------------------------------------------------
# Host-side preprocessing: shared program structure + per-core input tensors
# --------------------------------------------------------------------------

def host_prep(x, edge_index):
    import ml_dtypes

    x = np.asarray(x, np.float32)
    ei = np.asarray(edge_index)
    row = ei[0].astype(np.int64)
    col = ei[1].astype(np.int64)

    core = col // S
    per_core = []
    for c in range(N_CORES):
        m = core == c
        src = row[m]
        dloc = col[m] - c * S
        win = dloc // WIN
        slot = dloc % WIN
        buck = src // BR
        order = np.lexsort((src, win, buck))
        per_core.append((src[order], win[order], slot[order], buck[order]))

    # window-pure padded layout: each (bucket, window) run starts at a
    # common tile-aligned offset on every core, padded to the max run length
    # across cores (rounded to x128).  This removes cross-core span slop and
    # partial boundary tiles, roughly halving the scatter-matmul entry count
    # (NENT) — the PE instruction-issue rate is the compute-side bottleneck —
    # at the cost of ~19% more gather indices (pad indices repeat a valid
    # row, so the extra HBM reads mostly hit open rows).
    cnt = np.zeros((N_CORES, NBUCKET, NWIN), np.int64)
    for c in range(N_CORES):
        _, wins_c, _, bucks_c = per_core[c]
        np.add.at(cnt, (c, bucks_c, wins_c), 1)
    padded = (cnt.max(axis=0) + 127) // 128 * 128        # [NBUCKET, NWIN]
    ofs = np.zeros((NBUCKET, NWIN), np.int64)
    run = 0
    for b in range(NBUCKET):
        for w in range(NWIN):
            ofs[b, w] = run
            run += int(padded[b, w])
    NT = run // 128
    tiles_per_bucket = padded.sum(axis=1) // 128
    bucket_tile0 = np.concatenate([[0], np.cumsum(tiles_per_bucket)])[:NBUCKET]

    gidx = np.zeros((N_CORES, NT * 128), np.int16)      # within-bucket row idx
    mwin = np.full((N_CORES, NT * 128), -1, np.int32)   # window of each msg
    mslot = np.full((N_CORES, NT * 128), -1, np.int32)  # slot within window
    lo = np.zeros((N_CORES, NBUCKET, NWIN), np.int64)
    hi = np.zeros((N_CORES, NBUCKET, NWIN), np.int64)
    for c in range(N_CORES):
        srcs, wins, slots, bucks = per_core[c]
        pos0 = 0
        for b in range(NBUCKET):
            for w in range(NWIN):
                n = int(cnt[c, b, w])
                base = int(ofs[b, w])
                if n:
                    seg = slice(pos0, pos0 + n)
                    idxs = (srcs[seg] - b * BR).astype(np.int16)
                    gidx[c, base:base + n] = idxs
                    # pad with a repeat of the last valid index (open-row HBM
                    # reads); mwin stays -1 so padding never matches a window
                    gidx[c, base + n:base + int(padded[b, w])] = idxs[-1]
                    mwin[c, base:base + n] = wins[seg]
                    mslot[c, base:base + n] = slots[seg]
                    pos0 += n
                lo[c, b, w] = base
                hi[c, b, w] = base + (int(padded[b, w]) if n else 0)

    # entries: per window w, per bucket b: union tile span across cores
    entries = []
    entry_windows = []
    win_entry_ofs = [0]
    for w in range(NWIN):
        for b in range(NBUCKET):
            if hi[:, b, w].max() <= lo[:, b, w].min():
                continue
            t0 = int(lo[:, b, w].min()) // 128
            t1 = int((hi[:, b, w].max() + 127) // 128)
            bt0 = int(bucket_tile0[b])
            bt1 = bt0 + int(tiles_per_bucket[b])
            t0, t1 = max(t0, bt0), min(t1, bt1)
            entries.extend(range(t0, t1))
            entry_windows.extend([w] * (t1 - t0))
        win_entry_ofs.append(len(entries))
    entries = np.array(entries, np.int64)
    entry_windows = np.array(entry_windows, np.int64)

    # per-core slot matrix per entry, stored DOUBLED along the entry axis
    # ([128, NENT*2], columns 2e and 2e+1 equal).  The duplicate gives every
    # operand of the U-build is_equal a stride-1 x2 innermost dim, which
    # qualifies the op for the DVE 2x_1p perf mode (2x throughput); with the
    # plain [128, NENT] layout the slots operand's innermost dim is a
    # stride-0 broadcast, which forces 1x.
    pos = entries[None, :] * 128 + np.arange(128)[:, None]      # [128, NENT]
    slots_bf = np.empty((N_CORES, 128, 2 * len(entries)), ml_dtypes.bfloat16)
    for c in range(N_CORES):
        wmatch = mwin[c][pos] == entry_windows[None, :]
        sl = np.where(wmatch, mslot[c][pos], -1).astype(ml_dtypes.bfloat16)
        slots_bf[c] = np.repeat(sl, 2, axis=1)

    # gather calls: chop bucket tile spans into chunks, then interleave the
    # buckets by fractional progress so emission order matches consumption.
    calls = []
    for b in range(NBUCKET):
        t = int(bucket_tile0[b])
        end = t + int(tiles_per_bucket[b])
        while t < end:
            nt = min(CHUNK_TILES, end - t)
            frac = (t - int(bucket_tile0[b])) / max(1, int(tiles_per_bucket[b]))
            calls.append((frac, b, t, nt))
            t += nt
    calls.sort()
    calls = [(b, t, nt) for _, b, t, nt in calls]

    ewmax = int(np.max(np.diff(win_entry_ofs)))
    struct = dict(NT=NT, calls=calls, entries=entries,
                  entry_windows=entry_windows,
                  win_entry_ofs=np.array(win_entry_ofs, np.int64),
                  EWMAX=ewmax)

    iota_bf = np.tile(np.arange(128, dtype=np.float32),
                      (128, 1)).astype(ml_dtypes.bfloat16)
    per_core_ins = []
    for c in range(N_CORES):
        g = gidx[c]
        gw = np.tile(g.reshape(-1, 16).T, (8, 1)).copy()    # [128, NT*8]
        xt = np.empty((D + 1, S), np.float32)
        xt[:D] = x[c * S:(c + 1) * S].T
        xt[D] = 1.0
        per_core_ins.append(dict(gidx=gw, slots=np.ascontiguousarray(slots_bf[c]),
                                 xt=xt))
    return struct, per_core_ins, dict(iota=iota_bf)


# --------------------------------------------------------------------------
# Bass program
# --------------------------------------------------------------------------

def build_program(struct, reps=1, ablate="none"):
    from contextlib import ExitStack
    import concourse.tile as tile
    from concourse import bacc, mybir

    NT = struct["NT"]
    calls = struct["calls"]
    entries = struct["entries"]
    weo = struct["win_entry_ofs"]
    NENT = len(entries)
    EWMAX = struct["EWMAX"]

    nc = bacc.Bacc("TRN2", target_bir_lowering=False, debug=False,
                   num_swdge_queues=4)
    f32, bf16, i16 = mybir.dt.float32, mybir.dt.bfloat16, mybir.dt.int16
    Alu, Act, Ax = mybir.AluOpType, mybir.ActivationFunctionType, mybir.AxisListType

    xg = nc.dram_tensor("xg", [N_NODES, D], f32, kind="ExternalInput")
    gidx = nc.dram_tensor("gidx", [128, NT * 8], i16, kind="ExternalInput")
    slots = nc.dram_tensor("slots", [128, 2 * NENT], bf16, kind="ExternalInput")
    xt = nc.dram_tensor("xt", [D + 1, S], f32, kind="ExternalInput")
    w1b = nc.dram_tensor("w1b", [D + 1, OUT], f32, kind="ExternalInput")
    w2 = nc.dram_tensor("w2", [D, OUT], f32, kind="ExternalInput")
    gb = nc.dram_tensor("gb", [128, 2 * OUT], f32, kind="ExternalInput")
    iota = nc.dram_tensor("iota", [128, 128], bf16, kind="ExternalInput")
    out = nc.dram_tensor("out", [S, OUT], f32, kind="ExternalOutput")

    NBATCH = (NWIN + HBATCH - 1) // HBATCH

    tile2call = {}
    for ci, (b, t0, nt) in enumerate(calls):
        for t in range(t0, t0 + nt):
            tile2call[t] = (ci, t - t0)
    win_last_call = []
    running = -1
    for w in range(NWIN):
        ts = entries[weo[w]:weo[w + 1]]
        last = max((tile2call[int(t)][0] for t in ts), default=-1)
        running = max(running, last)
        win_last_call.append(running)

    with tile.TileContext(nc) as tc, ExitStack() as ctx:
        cpool = ctx.enter_context(tc.tile_pool(name="const", bufs=1))
        mpool = ctx.enter_context(tc.tile_pool(name="msgs", bufs=16))
        bpool = ctx.enter_context(tc.tile_pool(name="msgsbf", bufs=12))
        upool = ctx.enter_context(tc.tile_pool(name="umat", bufs=3))
        npool = ctx.enter_context(tc.tile_pool(name="nst", bufs=5, space="PSUM"))
        hpool = ctx.enter_context(tc.tile_pool(name="hps", bufs=3, space="PSUM"))
        spool = ctx.enter_context(tc.tile_pool(name="small", bufs=4))
        opool = ctx.enter_context(tc.tile_pool(name="outs", bufs=3))

        gidx_t = cpool.tile([128, NT * 8], i16)
        nc.sync.dma_start(out=gidx_t[:], in_=gidx.ap())
        slots_t = cpool.tile([128, 2 * NENT], bf16)
        nc.sync.dma_start(out=slots_t[:], in_=slots.ap())
        xt_t = cpool.tile([D + 1, S], f32)
        nc.sync.dma_start(out=xt_t[:], in_=xt.ap())
        w1b_t = cpool.tile([D + 1, OUT], f32)
        nc.sync.dma_start(out=w1b_t[:], in_=w1b.ap())
        w2_t = cpool.tile([D, OUT], f32)
        nc.sync.dma_start(out=w2_t[:], in_=w2.ap())
        gb_t = cpool.tile([128, 2 * OUT], f32)
        nc.sync.dma_start(out=gb_t[:], in_=gb.ap())
        iota_t = cpool.tile([128, 128], bf16)
        nc.sync.dma_start(out=iota_t[:], in_=iota.ap())
        eps_t = cpool.tile([128, 1], f32)
        nc.vector.memset(eps_t[:], EPS)

        def body():
            chunk_bf = {}
            next_call = [0]
            if ablate == "compute_only":
                # one real gather chunk; all entries read from it (timing
                # probe for the compute-side pipeline, numerically wrong)
                b, t0, nt = calls[0]
                msgs = mpool.tile([128, CHUNK_TILES, D], f32, tag="mchunk")
                nc.gpsimd.dma_gather(
                    out_ap=msgs[:, :nt, :],
                    in_ap=xg.ap()[b * BR:min((b + 1) * BR, N_NODES)],
                    idxs_ap=gidx_t[:, t0 * 8:(t0 + nt) * 8],
                    num_idxs=nt * 128, num_idxs_reg=nt * 128, elem_size=D,
                    queue_num=0, single_packet=False)
                mbf = bpool.tile([128, CHUNK_TILES, D], bf16, tag="bchunk")
                nc.scalar.activation(out=mbf[:, :nt, :], in_=msgs[:, :nt, :],
                                     func=Act.Copy)
                for ci in range(len(calls)):
                    chunk_bf[ci] = mbf
                next_call[0] = len(calls)
            if ablate == "gather_only":
                for ci in range(len(calls)):
                    b, t0, nt = calls[ci]
                    msgs = mpool.tile([128, CHUNK_TILES, D], f32, tag="mchunk")
                    nc.gpsimd.dma_gather(
                        out_ap=msgs[:, :nt, :],
                        in_ap=xg.ap()[b * BR:min((b + 1) * BR, N_NODES)],
                        idxs_ap=gidx_t[:, t0 * 8:(t0 + nt) * 8],
                        num_idxs=nt * 128, num_idxs_reg=nt * 128, elem_size=D,
                        queue_num=ci % 4, single_packet=False)
                return

            def emit_call(ci):
                b, t0, nt = calls[ci]
                q = ci % 4
                msgs = mpool.tile([128, CHUNK_TILES, D], f32, tag="mchunk")
                nc.gpsimd.dma_gather(
                    out_ap=msgs[:, :nt, :], in_ap=xg.ap()[b * BR:min((b + 1) * BR, N_NODES)],
                    idxs_ap=gidx_t[:, t0 * 8:(t0 + nt) * 8],
                    num_idxs=nt * 128, num_idxs_reg=nt * 128, elem_size=D,
                    queue_num=q, single_packet=False)
                mbf = bpool.tile([128, CHUNK_TILES, D], bf16, tag="bchunk")
                nc.scalar.activation(
                    out=mbf[:, :nt, :], in_=msgs[:, :nt, :], func=Act.Copy)
                chunk_bf[ci] = mbf

            for batch in range(NBATCH):
                w0 = batch * HBATCH
                wn = min(HBATCH, NWIN - w0)
                hps = hpool.tile([128, HBATCH, OUT], f32)
                for j in range(wn):
                    w = w0 + j
                    tgt = win_last_call[min(w + 5, NWIN - 1)]
                    while next_call[0] <= tgt:
                        emit_call(next_call[0])
                        next_call[0] += 1
                    e0, e1 = int(weo[w]), int(weo[w + 1])
                    ew = e1 - e0
                    nst = npool.tile([OUT, WIN], f32)
                    if ew > 0:
                        U = upool.tile([128, ew, 128], bf16, tag="U")
                        # every operand's innermost dim is stride-1 x2 so the
                        # DVE picks its 2x_1p perf mode (see host_prep)
                        nc.vector.tensor_tensor(
                            out=U[:].rearrange("p e (dh dl) -> p e dh dl",
                                               dl=2),
                            in0=slots_t[:, 2 * e0:2 * e1]
                                .rearrange("p (e dh dl) -> p e dh dl",
                                           dh=1, dl=2)
                                .broadcast_to([128, ew, 64, 2]),
                            in1=iota_t[:]
                                .rearrange("p (e dh dl) -> p e dh dl",
                                           e=1, dl=2)
                                .broadcast_to([128, ew, 64, 2]),
                            op=Alu.is_equal)
                        for i, te in enumerate(range(e0, e1)):
                            t = int(entries[te])
                            ci, toff = tile2call[t]
                            nc.tensor.matmul(
                                out=nst[:], lhsT=chunk_bf[ci][:, toff, :],
                                rhs=U[:, i, :],
                                start=(i == 0), stop=(i == ew - 1))
                    else:
                        nc.vector.memset(nst[:], 0.0)
                    nsts = spool.tile([OUT, WIN], f32, tag="nsts")
                    nc.scalar.activation(out=nsts[:], in_=nst[:], func=Act.Copy)
                    nw = min(WIN, S - w * WIN)
                    nc.tensor.matmul(out=hps[:, j, :], lhsT=nsts[:],
                                     rhs=w2_t[:], start=True, stop=False)
                    nc.tensor.matmul(out=hps[:nw, j, :],
                                     lhsT=xt_t[:, w * WIN:w * WIN + nw],
                                     rhs=w1b_t[:], start=False, stop=True)
                # ---- batched LayerNorm over [128, wn, OUT] ----
                red = spool.tile([128, 8, HBATCH], f32, tag="red")
                nmu = red[:, 0, :wn]
                msq = red[:, 1, :wn]
                musq = red[:, 2, :wn]
                var = red[:, 3, :wn]
                std = red[:, 4, :wn]
                rstd = red[:, 5, :wn]
                nmr = red[:, 6, :wn]
                nc.vector.tensor_reduce(out=nmu[:], in_=hps[:, :wn, :],
                                        axis=Ax.X, op=Alu.add, negate=True)
                sq = spool.tile([128, HBATCH, OUT], f32, tag="sq")
                nc.scalar.activation(out=sq[:, :wn, :], in_=hps[:, :wn, :],
                                     func=Act.Square)
                nc.vector.tensor_reduce(out=msq[:], in_=sq[:, :wn, :],
                                        axis=Ax.X, op=Alu.add)
                nc.vector.tensor_scalar(out=nmu[:], in0=nmu[:],
                                        scalar1=1.0 / OUT, scalar2=None,
                                        op0=Alu.mult)
                nc.vector.tensor_scalar(out=msq[:], in0=msq[:],
                                        scalar1=1.0 / OUT, scalar2=None,
                                        op0=Alu.mult)
                nc.vector.tensor_tensor(out=musq[:], in0=nmu[:], in1=nmu[:],
                                        op=Alu.mult)
                nc.vector.tensor_tensor(out=var[:], in0=msq[:], in1=musq[:],
                                        op=Alu.subtract)
                nc.scalar.activation(out=std[:], in_=var[:], func=Act.Sqrt,
                                     bias=eps_t[:])
                nc.vector.reciprocal(out=rstd[:], in_=std[:])
                nc.vector.tensor_tensor(out=nmr[:], in0=nmu[:], in1=rstd[:],
                                        op=Alu.mult)
                z = opool.tile([128, HBATCH, OUT], f32, tag="z")
                for j in range(wn):
                    nc.scalar.activation(out=z[:, j, :], in_=hps[:, j, :],
                                         func=Act.Identity,
                                         bias=nmr[:, j:j + 1],
                                         scale=rstd[:, j:j + 1])
                nc.vector.tensor_tensor(
                    out=z[:, :wn, :], in0=z[:, :wn, :],
                    in1=gb_t[:, :OUT].rearrange("p (o d) -> p o d", o=1)
                        .broadcast_to([128, wn, OUT]),
                    op=Alu.mult)
                nc.vector.tensor_tensor(
                    out=z[:, :wn, :], in0=z[:, :wn, :],
                    in1=gb_t[:, OUT:].rearrange("p (o d) -> p o d", o=1)
                        .broadcast_to([128, wn, OUT]),
                    op=Alu.add)
                nfull = wn if (w0 + wn) * WIN <= S else wn - 1
                if nfull > 0:
                    dst = out.ap()[w0 * WIN:(w0 + nfull) * WIN] \
                        .rearrange("(j p) d -> p j d", p=128)
                    nc.sync.dma_start(out=dst, in_=z[:, :nfull, :])
                if nfull < wn:
                    tail = S - (w0 + nfull) * WIN
                    nc.sync.dma_start(out=out.ap()[(w0 + nfull) * WIN:S],
                                      in_=z[:tail, nfull, :])

        if reps == 1:
            body()
        else:
            with tc.For_i(0, reps, 1,
                          hint_engines=(mybir.EngineType.PE,
                                        mybir.EngineType.DVE,
                                        mybir.EngineType.Pool,
                                        mybir.EngineType.Activation,
                                        mybir.EngineType.SP)):
                body()

    nc.compile()
    return nc


# --------------------------------------------------------------------------
# Entry point
# --------------------------------------------------------------------------

def make_inputs(x, W, b, gamma, beta, struct, per_core, shared):
    w1b_a = np.concatenate([np.asarray(W, np.float32)[:D],
                            np.asarray(b, np.float32)[None, :]], axis=0)
    w2_a = np.ascontiguousarray(np.asarray(W, np.float32)[D:])
    gb_a = np.concatenate([np.tile(np.asarray(gamma, np.float32), (128, 1)),
                           np.tile(np.asarray(beta, np.float32), (128, 1))],
                          axis=1)
    in_maps = []
    for c in range(N_CORES):
        in_maps.append(dict(
            xg=np.asarray(x, np.float32), gidx=per_core[c]["gidx"],
            slots=per_core[c]["slots"], xt=per_core[c]["xt"],
            w1b=w1b_a, w2=w2_a, gb=gb_a, iota=shared["iota"]))
    return in_maps


def kernel(x, edge_index, W, b, gamma, beta):
    from concourse.bass_utils import run_bass_kernel_spmd

    struct, per_core, shared = host_prep(x, edge_index)
    nc = build_program(struct)
    in_maps = make_inputs(x, W, b, gamma, beta, struct, per_core, shared)
    res = run_bass_kernel_spmd(nc, in_maps, core_ids=list(range(N_CORES)))
    out = np.concatenate([res.results[c]["out"] for c in range(N_CORES)],
                         axis=0)
    return out.astype(np.float32)

